# revision 1
# baseline (speedup 1.0000x reference)
"""Trainium2 Bass kernel for nn_DifferentiableSynth.

Self-contained: takes FULL inputs (15 scalars + noise[14.4M]), returns [1, 14.4M].
Strategy: shard time axis across 8 cores (1792 blocks of 1024 each, partition=block
layout [128, 14 chunks x 1024]). Host computes O(1) scalars, O(N/16) cumsum row
carries (bit-exact f32 emulation of XLA's blocked cumsum), and O(NBLK) biquad
tables; device computes all O(N) per-sample work: phase quantization + range
reduction + ACT sine, source mix, 3-tap FIR, modulated first-order scans
(2nd-order IIR via rotating-frame decomposition), table combine with folded
amp envelope.
"""
import numpy as np

SR = 48000
DUR = 300.0
N = 14400000
BLOCK = 1024
NBLK = 14063                 # real blocks (ceil(N/1024))
NCORE = 8
NGRP = 14                    # chunks (groups) per core
GBLK = 128                   # blocks per chunk = partitions
CBLK = NGRP * GBLK           # 1792 blocks per core
CSAMP = CBLK * BLOCK         # 1,835,008 samples per core
TOTBLK = NCORE * CBLK        # 14336 padded blocks
NROW = N // 16               # 900000 rows of 16
ROWS_PC = CSAMP // 16        # 114688 rows per core -> [128, 896]
F32 = np.float32

MAGIC = np.float32(12582912.0)       # 1.5*2^23
C2PI = np.float32(6.2831855)         # fl32(2*pi)
INV2PI = np.float32(1.0 / (2.0 * np.pi))


def _serial_scan_rows(x2d):
    out = np.empty_like(x2d)
    acc = np.zeros(x2d.shape[0], dtype=F32)
    for j in range(x2d.shape[1]):
        acc = (acc + x2d[:, j]).astype(F32)
        out[:, j] = acc
    return out


def _xla_cumsum_full(x, base=16):
    n = x.shape[0]
    xp = np.pad(x, (0, (-n) % base))
    inner = _serial_scan_rows(xp.reshape(-1, base))
    sums = inner[:, -1].copy()
    if sums.shape[0] <= base:
        outer = _serial_scan_rows(sums[None, :])[0]
    else:
        _, outer = _xla_cumsum_full(sums, base)
    outer_excl = np.concatenate([np.zeros(1, F32), outer[:-1]])
    full = (inner + outer_excl[:, None]).astype(F32).reshape(-1)[:n]
    return inner, full


def _adsr64(a_s, d_s, sus, r_s, idx):
    a = a_s * SR; d = d_s * SR; r = r_s * SR
    attack_end = a; decay_end = a + d; sustain_end = max(decay_end, N - r)
    t = idx.astype(np.float64)
    env = np.full(t.shape, sus)
    env = np.where(t < decay_end, 1.0 - (t - attack_end) / max(d, 1e-5) * (1.0 - sus), env)
    env = np.where(t >= sustain_end, sus * (1.0 - (t - sustain_end) / max(r, 1e-5)), env)
    env = np.where(t < attack_end, t / max(a, 1e-5), env)
    return np.clip(env, 0.0, 1.0)


def _host_precompute(scal, noise):
    """All host-side preparation. Returns per-core input dicts + meta."""
    import jax
    import jax.numpy as jnp
    cpu = jax.devices("cpu")[0]

    def sig32(x):
        return np.asarray(jax.device_put(jnp.float32(x), cpu))

    with jax.default_device(cpu):
        s = {k: jnp.float32(v) for k, v in scal.items()}
        sg = jax.nn.sigmoid

        def sc(v, dmin, dmax):
            return np.asarray((v - 0.0) / (1.0 - 0.0) * (dmax - dmin) + dmin)

        noise_mix = float(np.asarray(sg(s["noise_mix_raw"])))
        start_freq = np.asarray(sc(sg(s["start_freq_raw"]), 20.0, 8000.0))
        end_freq = np.asarray(sc(sg(s["end_freq_raw"]), 20.0, 8000.0))
        pitch_decay = np.asarray(sc(sg(s["pitch_decay_raw"]), 0.01, 2.0))
        amp_attack = float(np.asarray(sc(sg(s["amp_attack_raw"]), 0.001, 1.0)))
        amp_decay = float(np.asarray(sc(sg(s["amp_decay_raw"]), 0.01, 2.0)))
        amp_sustain = float(np.asarray(sg(s["amp_sustain_raw"])))
        amp_release = float(np.asarray(sc(sg(s["amp_release_raw"]), 0.01, 2.0)))
        cutoff_base = float(np.asarray(sc(sg(s["filter_cutoff_raw"]), 100.0, 12000.0)))
        filter_q = float(np.asarray(sc(sg(s["filter_q_raw"]), 0.707, 10.0)))
        env_amount = float(np.asarray(
            (jnp.tanh(s["filter_env_amount_raw"]) - (-1.0)) / 2.0 * 16000.0 + (-8000.0)))
        fe_attack = float(np.asarray(sc(sg(s["filt_env_attack_raw"]), 0.001, 1.0)))
        fe_decay = float(np.asarray(sc(sg(s["filt_env_decay_raw"]), 0.01, 1.0)))
        fe_sustain = float(np.asarray(sg(s["filt_env_sustain_raw"])))
        fe_release = float(np.asarray(sc(sg(s["filt_env_release_raw"]), 0.01, 1.0)))

        # ---- transient f values (f32 chain, f64 exp) ----
        tau32 = (np.asarray(pitch_decay).astype(F32) + F32(1e-6)).astype(F32)
        tau = float(tau32)
        i_star = int(np.ceil(-np.log(2.0 ** -26) * tau * (N - 1) / DUR)) + 4096
        trans_chunks = min(NGRP, (i_star // (GBLK * BLOCK)) + 1)
        TRANS = trans_chunks * GBLK * BLOCK
        t_f32 = np.asarray(jnp.linspace(0.0, DUR, N)[:TRANS])

    arg = (-t_f32 / tau32).astype(F32)
    pc = np.exp(arg.astype(np.float64)).astype(F32)
    one_m = (F32(1.0) - pc).astype(F32)
    dfreq = (end_freq.astype(F32) - start_freq.astype(F32)).astype(F32)
    freq_tr = (start_freq.astype(F32) + (dfreq * one_m).astype(F32)).astype(F32)
    f_tr = ((C2PI * freq_tr).astype(F32) / F32(SR)).astype(F32)
    cval = ((C2PI * (start_freq.astype(F32) + dfreq).astype(F32)).astype(F32)
            / F32(SR)).astype(F32)

    # ---- cumsum carries: row-of-16 inner prefixes + exclusive row carries ----
    f_full = np.full(N, cval, dtype=F32)
    f_full[:TRANS] = f_tr
    xp = f_full.reshape(-1, 16)
    inner0 = _serial_scan_rows(xp)               # [900000, 16]
    sums0 = inner0[:, -1].copy()
    _, S1 = _xla_cumsum_full(sums0)              # inclusive scan of row sums
    S1x = np.concatenate([np.zeros(1, F32), S1[:-1]])   # exclusive row carries

    # pad rows to 8*114688
    ROWS_TOT = NCORE * ROWS_PC
    S1x_pad = np.concatenate([S1x, np.zeros(ROWS_TOT - NROW, F32)])
    inner0_pad = np.concatenate([inner0, np.zeros((ROWS_TOT - NROW, 16), F32)], 0)

    # r_row (range-reduced carries, f64 precision) and binade magic per (p, g)
    S64 = S1x_pad.astype(np.float64)
    m_int = np.round(S64 * (1.0 / (2 * np.pi)) - 0.05)
    r_row = (S64 - m_int * (2 * np.pi)).astype(F32)

    # per-core row layout [128, 896]: row R(c,p,g,w) = ((c*14+g)*128+p)*64+w
    r4 = r_row.reshape(NCORE, NGRP, GBLK, 64)
    rrow_pc = np.ascontiguousarray(r4.transpose(0, 2, 1, 3)).reshape(NCORE, 128, NGRP * 64)
    S4 = S1x_pad.reshape(NCORE, NGRP, GBLK, 64)
    Sfirst = S4[:, :, :, 0]                       # [core, g, p]
    bits = Sfirst.view(np.uint32) if Sfirst.dtype == F32 else Sfirst.astype(F32).view(np.uint32)
    binade = (bits & np.uint32(0x7F800000)).view(F32)
    Mpg = (F32(1.5) * binade).astype(F32).transpose(0, 2, 1)   # [core, p, g]
    Mpg = np.ascontiguousarray(Mpg)

    # I0 for transient chunks (per-sample inner prefixes)
    tcn = max(2, trans_chunks)
    i4 = inner0_pad.reshape(NCORE, NGRP, GBLK, 64, 16)
    I0 = np.ascontiguousarray(i4[:, 0:tcn].transpose(0, 2, 1, 3, 4)).reshape(NCORE, 128, tcn * 1024)

    # P0rep [128, 1024]: const-row inner prefix pattern repeated 64x
    P0 = np.zeros(16, F32)
    acc = F32(0.0)
    for j in range(16):
        acc = F32(acc + cval)
        P0[j] = acc
    P0rep = np.tile(np.tile(P0, 64)[None, :], (128, 1))

    # ---- per-block filter/amp tables ----
    alpha_mix = 1.0 - noise_mix
    gamma = noise_mix / alpha_mix
    blk = np.arange(NBLK, dtype=np.int64)
    # cutoff_b: f64 mean of clip(cutoff_base + filt_env*env_amount) per block
    dec_end_b = int((fe_attack + fe_decay) * SR // BLOCK) + 2
    sus_start_b = int((N - fe_release * SR) // BLOCK) - 1
    cutoff_b = np.empty(NBLK, np.float64)
    fe_sus_cut = np.clip(cutoff_base + fe_sustain * env_amount, 20.0, SR / 2.1)
    cutoff_b[:] = fe_sus_cut
    vary_blocks = list(range(0, min(dec_end_b, NBLK))) + list(range(max(sus_start_b, 0), NBLK))
    for b in vary_blocks:
        idx = np.arange(b * BLOCK, (b + 1) * BLOCK)
        fe = _adsr64(fe_attack, fe_decay, fe_sustain, fe_release, idx)
        cutoff_b[b] = np.clip(cutoff_base + fe * env_amount, 20.0, SR / 2.1).mean()
    w0 = 2.0 * np.pi * cutoff_b / SR
    alpha_f = np.sin(w0) / (2.0 * filter_q)
    cosw = np.cos(w0)
    b0 = (1.0 - cosw) / 2.0
    a0e = 1.0 + alpha_f + 1e-8
    b0n = b0 / a0e
    a1n = (-2.0 * cosw) / a0e
    a2n = (1.0 - alpha_f) / a0e
    rr = np.sqrt(a2n)
    th = np.arccos(np.clip(-a1n / (2.0 * rr), -1.0, 1.0))
    sth = np.sin(th)

    tgrid = np.arange(BLOCK, dtype=np.float64)

    def mk_tables(bsel, amp_per_sample):
        """tables [len(bsel), 4, 1024]: CT, ST, AT, BT (amp folded into AT/BT)"""
        nb = len(bsel)
        out = np.empty((nb, 4, BLOCK), F32)
        for i, b in enumerate(bsel):
            if b >= NBLK:
                b = NBLK - 1   # pad blocks: any finite values
            ct = b0n[b] * alpha_mix * np.cos(th[b] * tgrid)
            st = b0n[b] * alpha_mix * np.sin(th[b] * tgrid)
            if amp_per_sample:
                idx = np.arange(b * BLOCK, (b + 1) * BLOCK)
                amp = _adsr64(amp_attack, amp_decay, amp_sustain, amp_release, idx)
            else:
                amp = amp_sustain
            at = amp * np.sin(th[b] * (tgrid + 1.0)) / sth[b]
            bt = -(amp * np.cos(th[b] * (tgrid + 1.0)) / sth[b])
            out[i, 0] = ct; out[i, 1] = st; out[i, 2] = at; out[i, 3] = bt
        return out

    # shared sustain tables (any sustain block index)
    bsus = dec_end_b + 8
    shared = mk_tables([bsus], False)[0]                      # [4, 1024]
    shared_tile = np.tile(shared.reshape(1, 4 * BLOCK), (128, 1))  # [128, 4096]

    # special chunk tables per core: chunk 0 on core 0, chunk 11 on core 7
    amp_dec_end_b = int((amp_attack + amp_decay) * SR // BLOCK) + 2
    tbl_g0 = np.tile(shared_tile[None], (NCORE, 1, 1)).copy()
    sel0 = list(range(0, 128))    # blocks 0..127 on core 0 chunk 0
    t0 = mk_tables(sel0, True).reshape(128, 4 * BLOCK)
    tbl_g0[0] = t0
    SPECIAL_G = 11
    tbl_gS = np.tile(shared_tile[None], (NCORE, 1, 1)).copy()
    base7 = (7 * NGRP + SPECIAL_G) * GBLK
    selS = [base7 + p for p in range(128)]
    tS = mk_tables(selS, True).reshape(128, 4 * BLOCK)
    tbl_gS[7] = tS

    # scan pole radius per (p, g) per core
    rcol = np.full((NCORE, 128, NGRP), rr[bsus], F32)
    for c in range(NCORE):
        for g in range(NGRP):
            gb = (c * NGRP + g) * GBLK
            bs = np.minimum(np.arange(gb, gb + GBLK), NBLK - 1)
            rcol[c, :, g] = rr[bs].astype(F32)

    # noise shards [core, 128, 14336]
    noise_pad = np.concatenate([noise.astype(F32), np.zeros(TOTBLK * BLOCK - N, F32)])
    nz = np.ascontiguousarray(
        noise_pad.reshape(NCORE, NGRP, GBLK, BLOCK).transpose(0, 2, 1, 3)
    ).reshape(NCORE, 128, NGRP * BLOCK)

    in_maps = []
    for c in range(NCORE):
        in_maps.append({
            "nz": nz[c],
            "rrow": np.ascontiguousarray(rrow_pc[c]),
            "mpg": np.ascontiguousarray(Mpg[c]),
            "mpgn": np.ascontiguousarray((-Mpg[c]).astype(F32)),
            "rcol": np.ascontiguousarray(rcol[c]),
            "p0rep": P0rep,
            "i0": np.ascontiguousarray(I0[c]),
            "tblS": shared_tile,
            "tbl0": np.ascontiguousarray(tbl_g0[c]),
            "tblB": np.ascontiguousarray(tbl_gS[c]),
        })
    meta = {"gamma": gamma, "trans_chunks": tcn, "special_g": SPECIAL_G}
    return in_maps, meta


def _build_kernel(gamma, trans_chunks, special_g):
    from contextlib import ExitStack
    import concourse.bass as bass
    import concourse.tile as tile
    from concourse import bacc, mybir

    A = mybir.AluOpType
    DT = mybir.dt.float32
    P = 128
    FB = BLOCK

    nc = bacc.Bacc("TRN2", target_bir_lowering=False, debug=False, num_devices=NCORE)
    d_nz = nc.dram_tensor("nz", [P, NGRP * FB], DT, kind="ExternalInput").ap()
    d_rrow = nc.dram_tensor("rrow", [P, NGRP * 64], DT, kind="ExternalInput").ap()
    d_mpg = nc.dram_tensor("mpg", [P, NGRP], DT, kind="ExternalInput").ap()
    d_mpgn = nc.dram_tensor("mpgn", [P, NGRP], DT, kind="ExternalInput").ap()
    d_rcol = nc.dram_tensor("rcol", [P, NGRP], DT, kind="ExternalInput").ap()
    d_p0 = nc.dram_tensor("p0rep", [P, FB], DT, kind="ExternalInput").ap()
    d_i0 = nc.dram_tensor("i0", [P, trans_chunks * FB], DT, kind="ExternalInput").ap()
    d_tblS = nc.dram_tensor("tblS", [P, 4 * FB], DT, kind="ExternalInput").ap()
    d_tbl0 = nc.dram_tensor("tbl0", [P, 4 * FB], DT, kind="ExternalInput").ap()
    d_tblB = nc.dram_tensor("tblB", [P, 4 * FB], DT, kind="ExternalInput").ap()
    d_out = nc.dram_tensor("out", [P, NGRP * FB], DT, kind="ExternalOutput").ap()

    with tile.TileContext(nc) as tc, ExitStack() as ctx:
        statics = ctx.enter_context(tc.tile_pool(name="static", bufs=1))
        work = ctx.enter_context(tc.tile_pool(name="work", bufs=3))

        rrow = statics.tile([P, NGRP * 64], DT)
        mpg = statics.tile([P, NGRP], DT)
        mpgn = statics.tile([P, NGRP], DT)
        rcolt = statics.tile([P, NGRP], DT)
        p0 = statics.tile([P, FB], DT)
        i0t = statics.tile([P, trans_chunks * FB], DT)
        tblS = statics.tile([P, 4 * FB], DT)
        tbl0 = statics.tile([P, 4 * FB], DT)
        tblB = statics.tile([P, 4 * FB], DT)
        nc.sync.dma_start(rrow[:], d_rrow[:])
        nc.sync.dma_start(mpg[:], d_mpg[:])
        nc.sync.dma_start(mpgn[:], d_mpgn[:])
        nc.sync.dma_start(rcolt[:], d_rcol[:])
        nc.sync.dma_start(p0[:], d_p0[:])
        nc.sync.dma_start(i0t[:], d_i0[:])
        nc.sync.dma_start(tblS[:], d_tblS[:])
        nc.sync.dma_start(tbl0[:], d_tbl0[:])
        nc.sync.dma_start(tblB[:], d_tblB[:])
        sinbias = statics.tile([P, 1], DT)
        nc.vector.memset(sinbias[:], 0.0)
        negmagic = statics.tile([P, 1], DT)
        nc.vector.memset(negmagic[:], -float(MAGIC))
        wbufs = []
        for _wi in range(3):
            _wt = statics.tile([P, FB + 2], DT, tag=f"wb{_wi}")
            nc.vector.memset(_wt[:, 0:2], 0.0)
            wbufs.append(_wt)

        def front(g):
            sl = slice(g * FB, (g + 1) * FB)
            nz = work.tile([P, FB], DT, tag="nz")
            nc.sync.dma_start(nz[:], d_nz[:, sl])
            src = i0t[:, sl] if g < trans_chunks else p0[:]
            t1 = work.tile([P, FB], DT, tag="t1")
            nc.scalar.activation(t1[:], src, mybir.ActivationFunctionType.Identity,
                                 bias=mpg[:, g:g + 1])
            nc.scalar.activation(t1[:], t1[:], mybir.ActivationFunctionType.Identity,
                                 bias=mpgn[:, g:g + 1])
            rbx = work.tile([P, FB], DT, tag="rbx")
            rb_ap = rrow[:, g * 64:(g + 1) * 64].rearrange(
                "p (w j) -> p w j", j=1).broadcast_to([P, 64, 16])
            nc.scalar.activation(
                rbx[:].rearrange("p (w j) -> p w j", w=64), rb_ap,
                mybir.ActivationFunctionType.Copy)
            nc.vector.tensor_tensor(rbx[:], t1[:], rbx[:], A.add)   # ph
            qp = work.tile([P, FB], DT, tag="qp")
            nc.scalar.activation(qp[:], rbx[:], mybir.ActivationFunctionType.Copy,
                                 bias=float(MAGIC), scale=float(INV2PI))
            nc.scalar.activation(qp[:], qp[:], mybir.ActivationFunctionType.Identity,
                                 bias=negmagic[:])
            nc.scalar.activation(qp[:], qp[:], mybir.ActivationFunctionType.Identity,
                                 bias=sinbias[:], scale=float(C2PI))
            nc.vector.tensor_tensor(rbx[:], rbx[:], qp[:], A.subtract)  # p1
            sine = work.tile([P, FB], DT, tag="sine")
            nc.scalar.activation(sine[:], rbx[:], mybir.ActivationFunctionType.Sin,
                                 bias=sinbias[:])
            w = wbufs[g % 3]
            nc.vector.scalar_tensor_tensor(w[:, 2:FB + 2], nz[:], float(gamma),
                                           sine[:], A.mult, A.add)
            e1 = work.tile([P, FB], DT, tag="e1")
            nc.vector.tensor_tensor(e1[:], w[:, 2:FB + 2], w[:, 0:FB], A.add)
            nc.vector.scalar_tensor_tensor(e1[:], w[:, 1:FB + 1], 2.0, e1[:],
                                           A.mult, A.add)
            return g, e1

        def back(g, e1):
            sl = slice(g * FB, (g + 1) * FB)
            tb = tbl0 if g == 0 else (tblB if g == special_g else tblS)
            d1 = work.tile([P, FB], DT, tag="d1")
            d2 = work.tile([P, FB], DT, tag="d2")
            nc.vector.tensor_tensor(d1[:], e1[:], tb[:, 0:FB], A.mult)
            nc.vector.tensor_tensor(d2[:], e1[:], tb[:, FB:2 * FB], A.mult)
            S1 = work.tile([P, FB], DT, tag="S1")
            S2 = work.tile([P, FB], DT, tag="S2")
            rb = rcolt[:, g:g + 1].broadcast_to([P, FB])
            nc.vector.tensor_tensor_scan(S1[:], rb, d1[:], 0.0, A.mult, A.add)
            nc.vector.tensor_tensor_scan(S2[:], rb, d2[:], 0.0, A.mult, A.add)
            nc.vector.tensor_tensor(S1[:], S1[:], tb[:, 2 * FB:3 * FB], A.mult)
            nc.vector.tensor_tensor(S2[:], S2[:], tb[:, 3 * FB:4 * FB], A.mult)
            nc.vector.tensor_tensor(S1[:], S1[:], S2[:], A.add)
            nc.sync.dma_start(d_out[:, sl], S1[:])

        from collections import deque
        pend = deque()
        for g in range(NGRP):
            pend.append(front(g))
            if len(pend) > 2:
                back(*pend.popleft())
        while pend:
            back(*pend.popleft())
    nc.compile()
    return nc


_CACHE = {}
_TRACE = False
_LAST_RES = None


def kernel(**inputs):
    noise = np.asarray(inputs["noise"], dtype=F32)
    scal = {k: float(np.asarray(v)) for k, v in inputs.items() if k != "noise"}
    in_maps, meta = _host_precompute(scal, noise)

    key = "nc"
    if key not in _CACHE:
        _CACHE[key] = _build_kernel(meta["gamma"], meta["trans_chunks"],
                                    meta["special_g"])
    nc = _CACHE[key]

    from concourse.bass_utils import run_bass_kernel_spmd
    res = run_bass_kernel_spmd(nc, in_maps, list(range(NCORE)), trace=_TRACE)
    globals()["_LAST_RES"] = res
    out = np.empty((NCORE, 128, NGRP, BLOCK), F32)
    for c in range(NCORE):
        out[c] = res.results[c]["out"].reshape(128, NGRP, BLOCK)
    full = out.transpose(0, 2, 1, 3).reshape(-1)[:N]
    return full[None, :]



# revision 14
# speedup vs baseline: 1.6145x; 1.6145x over previous
"""Trainium2 Bass kernel for nn_DifferentiableSynth.

Self-contained: takes FULL inputs (15 scalars + noise[14.4M]), returns [1, 14.4M].
Strategy: shard time axis across 8 cores (1792 blocks of 1024 each, partition=block
layout [128, 14 chunks x 1024]). Host computes O(1) scalars, O(N/16) cumsum row
carries (bit-exact f32 emulation of XLA's blocked cumsum), and exact f32 emulation
of the ~163 transient/release blocks (head ADSR + tail release), which are
overwritten after the gather; device computes all steady-state O(N) work: phase
quantization + range reduction + ACT sine, source mix, then the per-block biquad
as a 512-tap truncated-FIR via PE transposes + fp32r Toeplitz matmuls (amp and
(1-mix) gains folded into the FIR taps).
"""
import numpy as np

SR = 48000
DUR = 300.0
N = 14400000
BLOCK = 1024
NBLK = 14063                 # real blocks (ceil(N/1024))
NCORE = 8
NGRP = 14                    # chunks (groups) per core
GBLK = 128                   # blocks per chunk = partitions
CBLK = NGRP * GBLK           # 1792 blocks per core
CSAMP = CBLK * BLOCK         # 1,835,008 samples per core
TOTBLK = NCORE * CBLK        # 14336 padded blocks
NROW = N // 16               # 900000 rows of 16
ROWS_PC = CSAMP // 16        # 114688 rows per core -> [128, 896]
F32 = np.float32
TAPS = 512                   # FIR truncation (pole radius^512 << tol)
NLAG = TAPS // 128           # 4 lag sub-matrices

MAGIC = np.float32(12582912.0)       # 1.5*2^23
C2PI = np.float32(6.2831855)         # fl32(2*pi)
INV2PI = np.float32(1.0 / (2.0 * np.pi))


def _serial_scan_rows(x2d):
    out = np.empty_like(x2d)
    acc = np.zeros(x2d.shape[0], dtype=F32)
    for j in range(x2d.shape[1]):
        acc = (acc + x2d[:, j]).astype(F32)
        out[:, j] = acc
    return out


def _xla_cumsum_full(x, base=16):
    n = x.shape[0]
    xp = np.pad(x, (0, (-n) % base))
    inner = _serial_scan_rows(xp.reshape(-1, base))
    sums = inner[:, -1].copy()
    if sums.shape[0] <= base:
        outer = _serial_scan_rows(sums[None, :])[0]
    else:
        _, outer = _xla_cumsum_full(sums, base)
    outer_excl = np.concatenate([np.zeros(1, F32), outer[:-1]])
    full = (inner + outer_excl[:, None]).astype(F32).reshape(-1)[:n]
    return inner, full


def _adsr64(a_s, d_s, sus, r_s, idx):
    a = a_s * SR; d = d_s * SR; r = r_s * SR
    attack_end = a; decay_end = a + d; sustain_end = max(decay_end, N - r)
    t = idx.astype(np.float64)
    env = np.full(t.shape, sus)
    env = np.where(t < decay_end, 1.0 - (t - attack_end) / max(d, 1e-5) * (1.0 - sus), env)
    env = np.where(t >= sustain_end, sus * (1.0 - (t - sustain_end) / max(r, 1e-5)), env)
    env = np.where(t < attack_end, t / max(a, 1e-5), env)
    return np.clip(env, 0.0, 1.0)


def _biquad_coeffs(cutoff, q):
    """f64 normalized biquad low-pass coeffs (b0,b1,b2,a1,a2), a0 += 1e-8."""
    w0 = 2.0 * np.pi * cutoff / SR
    alpha_f = np.sin(w0) / (2.0 * q)
    cosw = np.cos(w0)
    b0 = (1.0 - cosw) / 2.0
    a0e = 1.0 + alpha_f + 1e-8
    return (b0 / a0e, (1.0 - cosw) / a0e, b0 / a0e,
            (-2.0 * cosw) / a0e, (1.0 - alpha_f) / a0e)


def _host_precompute(scal, noise):
    """All host-side preparation. Returns per-core input dicts + meta + patches."""
    import jax
    import jax.numpy as jnp
    cpu = jax.devices("cpu")[0]

    with jax.default_device(cpu):
        s = {k: jnp.float32(v) for k, v in scal.items()}
        sg = jax.nn.sigmoid

        def sc(v, dmin, dmax):
            return np.asarray((v - 0.0) / (1.0 - 0.0) * (dmax - dmin) + dmin)

        noise_mix = float(np.asarray(sg(s["noise_mix_raw"])))
        start_freq = np.asarray(sc(sg(s["start_freq_raw"]), 20.0, 8000.0))
        end_freq = np.asarray(sc(sg(s["end_freq_raw"]), 20.0, 8000.0))
        pitch_decay = np.asarray(sc(sg(s["pitch_decay_raw"]), 0.01, 2.0))
        amp_attack = float(np.asarray(sc(sg(s["amp_attack_raw"]), 0.001, 1.0)))
        amp_decay = float(np.asarray(sc(sg(s["amp_decay_raw"]), 0.01, 2.0)))
        amp_sustain = float(np.asarray(sg(s["amp_sustain_raw"])))
        amp_release = float(np.asarray(sc(sg(s["amp_release_raw"]), 0.01, 2.0)))
        cutoff_base = float(np.asarray(sc(sg(s["filter_cutoff_raw"]), 100.0, 12000.0)))
        filter_q = float(np.asarray(sc(sg(s["filter_q_raw"]), 0.707, 10.0)))
        env_amount = float(np.asarray(
            (jnp.tanh(s["filter_env_amount_raw"]) - (-1.0)) / 2.0 * 16000.0 + (-8000.0)))
        fe_attack = float(np.asarray(sc(sg(s["filt_env_attack_raw"]), 0.001, 1.0)))
        fe_decay = float(np.asarray(sc(sg(s["filt_env_decay_raw"]), 0.01, 1.0)))
        fe_sustain = float(np.asarray(sg(s["filt_env_sustain_raw"])))
        fe_release = float(np.asarray(sc(sg(s["filt_env_release_raw"]), 0.01, 1.0)))

        # ---- transient f values (f32 chain, f64 exp) ----
        tau32 = (np.asarray(pitch_decay).astype(F32) + F32(1e-6)).astype(F32)
        tau = float(tau32)
        i_star = int(np.ceil(-np.log(2.0 ** -26) * tau * (N - 1) / DUR)) + 4096
        trans_chunks = min(NGRP, (i_star // (GBLK * BLOCK)) + 1)
        TRANS = trans_chunks * GBLK * BLOCK
        t_f32 = np.asarray(jnp.linspace(0.0, DUR, N)[:TRANS])

    arg = (-t_f32 / tau32).astype(F32)
    pc = np.exp(arg.astype(np.float64)).astype(F32)
    one_m = (F32(1.0) - pc).astype(F32)
    dfreq = (end_freq.astype(F32) - start_freq.astype(F32)).astype(F32)
    freq_tr = (start_freq.astype(F32) + (dfreq * one_m).astype(F32)).astype(F32)
    f_tr = ((C2PI * freq_tr).astype(F32) / F32(SR)).astype(F32)
    cval = ((C2PI * (start_freq.astype(F32) + dfreq).astype(F32)).astype(F32)
            / F32(SR)).astype(F32)

    # ---- cumsum carries: row-of-16 inner prefixes + exclusive row carries ----
    f_full = np.full(N, cval, dtype=F32)
    f_full[:TRANS] = f_tr
    xp = f_full.reshape(-1, 16)
    inner0 = _serial_scan_rows(xp)               # [900000, 16]
    sums0 = inner0[:, -1].copy()
    _, S1 = _xla_cumsum_full(sums0)              # inclusive scan of row sums
    S1x = np.concatenate([np.zeros(1, F32), S1[:-1]])   # exclusive row carries

    # pad rows to 8*114688
    ROWS_TOT = NCORE * ROWS_PC
    S1x_pad = np.concatenate([S1x, np.zeros(ROWS_TOT - NROW, F32)])
    inner0_pad = np.concatenate([inner0, np.zeros((ROWS_TOT - NROW, 16), F32)], 0)

    # r_row (range-reduced carries, f64 precision) and binade magic per (p, g)
    S64 = S1x_pad.astype(np.float64)
    m_int = np.round(S64 * (1.0 / (2 * np.pi)) - 0.05)
    r_row = (S64 - m_int * (2 * np.pi)).astype(F32)

    # per-core row layout [128, 896]: row R(c,p,g,w) = ((c*14+g)*128+p)*64+w
    r4 = r_row.reshape(NCORE, NGRP, GBLK, 64)
    rrow_pc = np.ascontiguousarray(r4.transpose(0, 2, 1, 3)).reshape(NCORE, 128, NGRP * 64)
    S4 = S1x_pad.reshape(NCORE, NGRP, GBLK, 64)
    Sfirst = S4[:, :, :, 0]                       # [core, g, p]
    bits = Sfirst.view(np.uint32) if Sfirst.dtype == F32 else Sfirst.astype(F32).view(np.uint32)
    binade = (bits & np.uint32(0x7F800000)).view(F32)
    Mpg = (F32(1.5) * binade).astype(F32).transpose(0, 2, 1)   # [core, p, g]
    Mpg = np.ascontiguousarray(Mpg)

    # I0 for transient chunks (per-sample inner prefixes)
    tcn = max(2, trans_chunks)
    i4 = inner0_pad.reshape(NCORE, NGRP, GBLK, 64, 16)
    I0 = np.ascontiguousarray(i4[:, 0:tcn].transpose(0, 2, 1, 3, 4)).reshape(NCORE, 128, tcn * 1024)

    # P0rep [128, 1024]: const-row inner prefix pattern repeated 64x
    P0 = np.zeros(16, F32)
    acc = F32(0.0)
    for j in range(16):
        acc = F32(acc + cval)
        P0[j] = acc
    P0rep = np.tile(np.tile(P0, 64)[None, :], (128, 1))

    # ---- sustain-region FIR taps (amp + (1-mix) folded in) ----
    alpha_mix = 1.0 - noise_mix
    gamma = noise_mix / alpha_mix
    fe_sus_cut = np.clip(cutoff_base + fe_sustain * env_amount, 20.0, SR / 2.1)
    b0n, b1n, b2n, a1n, a2n = _biquad_coeffs(fe_sus_cut, filter_q)
    h = np.zeros(TAPS, np.float64)
    y1 = y2 = 0.0
    for n in range(TAPS):
        fir = (b0n if n == 0 else 0.0) + (b1n if n == 1 else 0.0) + (b2n if n == 2 else 0.0)
        y = fir - a1n * y1 - a2n * y2
        h[n] = y
        y2 = y1; y1 = y
    h *= alpha_mix * amp_sustain
    # Hcat[tau, 128*l + t] = h[128*l + t - tau] (0 for negative lag)
    import ml_dtypes
    BF16 = ml_dtypes.bfloat16
    lag = (np.arange(TAPS)[None, :] - np.arange(128)[:, None])   # [tau, col]
    val = np.where((lag >= 0) & (lag < TAPS), h[np.clip(lag, 0, TAPS - 1)], 0.0)
    Hcat = val.astype(BF16)
    eye = np.eye(128, dtype=BF16)

    # ---- special (non-sustain) blocks: exact f32 host emulation ----
    dec_end_b = int((fe_attack + fe_decay) * SR // BLOCK) + 2
    amp_dec_end_b = int((amp_attack + amp_decay) * SR // BLOCK) + 2
    n_head = min(GBLK, max(dec_end_b, amp_dec_end_b, 2))
    amp_rel_start_b = int((N - amp_release * SR) // BLOCK)
    fe_rel_start_b = int((N - fe_release * SR) // BLOCK)
    tail_start = min(amp_rel_start_b, fe_rel_start_b, NBLK - 1)
    head_blocks = list(range(0, n_head))
    tail_blocks = list(range(tail_start, NBLK))

    def emulate(blist):
        nb = len(blist)
        barr = np.array(blist, np.int64)
        rows = (barr[:, None] * 64 + np.arange(64)[None, :]).reshape(-1)   # rows of 16
        ph32 = (S1x_pad[rows][:, None] + inner0_pad[rows]).astype(F32).reshape(nb, BLOCK)
        sine = np.sin(ph32.astype(np.float64)).astype(F32)
        nzb = np.zeros((nb, BLOCK), F32)
        for i, b in enumerate(blist):
            s0, s1 = b * BLOCK, min((b + 1) * BLOCK, N)
            nzb[i, :s1 - s0] = noise[s0:s1]
        src = ((F32(alpha_mix) * sine).astype(F32) + (F32(noise_mix) * nzb).astype(F32)).astype(F32)
        # per-block cutoff mean (f64 adsr; pad-aware for last block)
        co = np.empty(nb, np.float64)
        for i, b in enumerate(blist):
            idx = np.arange(b * BLOCK, (b + 1) * BLOCK)
            fe = _adsr64(fe_attack, fe_decay, fe_sustain, fe_release, idx)
            cut = np.clip(cutoff_base + fe * env_amount, 20.0, SR / 2.1)
            cut = np.where(idx < N, cut, 0.0)       # reference pads cutoff with 0
            co[i] = cut.mean()
        cb0, cb1, cb2, ca1, ca2 = _biquad_coeffs(co, filter_q)
        cb0 = cb0.astype(F32)[:, None]; cb1 = cb1.astype(F32)[:, None]
        cb2 = cb2.astype(F32)[:, None]
        ca1 = ca1.astype(F32); ca2 = ca2.astype(F32)
        x1 = np.zeros_like(src); x1[:, 1:] = src[:, :-1]
        x2 = np.zeros_like(src); x2[:, 2:] = src[:, :-2]
        fir = ((cb0 * src).astype(F32) + (cb1 * x1).astype(F32)).astype(F32)
        fir = (fir + (cb2 * x2).astype(F32)).astype(F32)
        y = np.zeros((nb, BLOCK), F32)
        yy1 = np.zeros(nb, F32); yy2 = np.zeros(nb, F32)
        for t in range(BLOCK):
            v = ((fir[:, t] - (ca1 * yy1).astype(F32)).astype(F32)
                 - (ca2 * yy2).astype(F32)).astype(F32)
            y[:, t] = v
            yy2 = yy1; yy1 = v
        for i, b in enumerate(blist):
            idx = np.arange(b * BLOCK, (b + 1) * BLOCK)
            amp = _adsr64(amp_attack, amp_decay, amp_sustain, amp_release, idx).astype(F32)
            y[i] = (y[i] * amp).astype(F32)
        return y

    patches = []
    for blist in (head_blocks, tail_blocks):
        if not blist:
            continue
        yv = emulate(blist)
        for i, b in enumerate(blist):
            s0, s1 = b * BLOCK, min((b + 1) * BLOCK, N)
            patches.append((s0, yv[i, :s1 - s0]))

    # noise shards [core, 128, 14336]
    noise_pad = np.concatenate([noise.astype(F32), np.zeros(TOTBLK * BLOCK - N, F32)])
    nz = np.ascontiguousarray(
        noise_pad.reshape(NCORE, NGRP, GBLK, BLOCK).transpose(0, 2, 1, 3)
    ).reshape(NCORE, 128, NGRP * BLOCK)

    in_maps = []
    for c in range(NCORE):
        in_maps.append({
            "nz": nz[c],
            "rrow": np.ascontiguousarray(rrow_pc[c]),
            "mpg": np.ascontiguousarray(Mpg[c]),
            "mpgn": np.ascontiguousarray((-Mpg[c]).astype(F32)),
            "p0rep": P0rep,
            "i0": np.ascontiguousarray(I0[c]),
            "hcat": Hcat,
            "eye": eye,
        })
    meta = {"gamma": gamma, "trans_chunks": tcn}
    return in_maps, meta, patches


def _build_kernel(gamma, trans_chunks):
    from contextlib import ExitStack
    import concourse.bass as bass
    import concourse.tile as tile
    from concourse import bacc, mybir

    A = mybir.AluOpType
    DT = mybir.dt.float32
    BF = mybir.dt.bfloat16
    P = 128
    FB = BLOCK

    nc = bacc.Bacc("TRN2", target_bir_lowering=False, debug=False, num_devices=NCORE)
    d_nz = nc.dram_tensor("nz", [P, NGRP * FB], DT, kind="ExternalInput").ap()
    d_rrow = nc.dram_tensor("rrow", [P, NGRP * 64], DT, kind="ExternalInput").ap()
    d_mpg = nc.dram_tensor("mpg", [P, NGRP], DT, kind="ExternalInput").ap()
    d_mpgn = nc.dram_tensor("mpgn", [P, NGRP], DT, kind="ExternalInput").ap()
    d_p0 = nc.dram_tensor("p0rep", [P, FB], DT, kind="ExternalInput").ap()
    d_i0 = nc.dram_tensor("i0", [P, trans_chunks * FB], DT, kind="ExternalInput").ap()
    d_hcat = nc.dram_tensor("hcat", [P, TAPS], BF, kind="ExternalInput").ap()
    d_eye = nc.dram_tensor("eye", [P, P], BF, kind="ExternalInput").ap()
    d_out = nc.dram_tensor("out", [P, NGRP * FB], DT, kind="ExternalOutput").ap()

    with tile.TileContext(nc) as tc, ExitStack() as ctx:
        statics = ctx.enter_context(tc.tile_pool(name="static", bufs=1))
        work = ctx.enter_context(tc.tile_pool(name="work", bufs=3))
        psum = ctx.enter_context(tc.tile_pool(name="ps", bufs=2, space="PSUM"))

        rrow = statics.tile([P, NGRP * 64], DT)
        mpg = statics.tile([P, NGRP], DT)
        mpgn = statics.tile([P, NGRP], DT)
        p0 = statics.tile([P, FB], DT)
        i0t = statics.tile([P, trans_chunks * FB], DT)
        hcat = statics.tile([P, TAPS], BF)
        eye = statics.tile([P, P], BF)
        nc.sync.dma_start(rrow[:], d_rrow[:])
        nc.sync.dma_start(mpg[:], d_mpg[:])
        nc.sync.dma_start(mpgn[:], d_mpgn[:])
        nc.sync.dma_start(p0[:], d_p0[:])
        nc.sync.dma_start(i0t[:], d_i0[:])
        nc.sync.dma_start(hcat[:], d_hcat[:])
        nc.sync.dma_start(eye[:], d_eye[:])
        sinbias = statics.tile([P, 1], DT)
        nc.vector.memset(sinbias[:], 0.0)
        magict = statics.tile([P, 1], DT)
        nc.vector.memset(magict[:], float(MAGIC))
        c2pit = statics.tile([P, 1], DT)
        nc.vector.memset(c2pit[:], float(C2PI))

        def front(g):
            sl = slice(g * FB, (g + 1) * FB)
            nz = work.tile([P, FB], DT, tag="nz")
            nc.sync.dma_start(nz[:], d_nz[:, sl])
            src_q = i0t[:, sl] if g < trans_chunks else p0[:]
            t1 = work.tile([P, FB], DT, tag="t1")
            nc.scalar.activation(t1[:], src_q, mybir.ActivationFunctionType.Identity,
                                 bias=mpg[:, g:g + 1])
            nc.scalar.activation(t1[:], t1[:], mybir.ActivationFunctionType.Identity,
                                 bias=mpgn[:, g:g + 1])
            ph = work.tile([P, FB], DT, tag="ph")
            rb_ap = rrow[:, g * 64:(g + 1) * 64].rearrange(
                "p (w j) -> p w j", j=1).broadcast_to([P, 64, 16])
            nc.gpsimd.tensor_tensor(
                ph[:].rearrange("p (w j) -> p w j", w=64),
                t1[:].rearrange("p (w j) -> p w j", w=64), rb_ap, A.add)
            qp = work.tile([P, FB], DT, tag="qp")
            nc.scalar.activation(qp[:], ph[:], mybir.ActivationFunctionType.Copy,
                                 bias=float(MAGIC), scale=float(INV2PI))
            nc.gpsimd.tensor_tensor(qp[:], qp[:], magict[:].broadcast_to([P, FB]),
                                    A.subtract)
            nc.gpsimd.tensor_tensor(qp[:], qp[:], c2pit[:].broadcast_to([P, FB]),
                                    A.mult)
            p1 = work.tile([P, FB], DT, tag="p1")
            nc.gpsimd.tensor_tensor(p1[:], ph[:], qp[:], A.subtract)
            sine = work.tile([P, FB], DT, tag="sine")
            nc.scalar.activation(sine[:], p1[:], mybir.ActivationFunctionType.Sin,
                                 bias=sinbias[:])
            src = work.tile([P, FB], BF, tag="src")
            nc.vector.scalar_tensor_tensor(src[:], nz[:], float(gamma),
                                           sine[:], A.mult, A.add)
            return src

        def back(g, src):
            sl = slice(g * FB, (g + 1) * FB)
            xt_ps = psum.tile([P, FB], BF, tag="xt")
            for k in range(8):
                nc.tensor.transpose(xt_ps[:, k * 128:(k + 1) * 128],
                                    src[:, k * 128:(k + 1) * 128], eye[:])
            xt = work.tile([P, FB], BF, tag="xtsb")
            nc.vector.tensor_copy(out=xt[:], in_=xt_ps[:])
            xtr = xt[:]
            hcr = hcat[:]
            y0 = psum.tile([P, 512], DT, tag="y0")     # out cols 0..511
            y1 = psum.tile([P, 512], DT, tag="y1")     # out cols 512..1023
            # k=0 full write to bank A; k=4 full write to bank B (start=True)
            nc.tensor.matmul(y0[:], xtr[:, 0:128], hcr[:, 0:512],
                             start=True, stop=False)
            nc.tensor.matmul(y1[:], xtr[:, 512:640], hcr[:, 0:512],
                             start=True, stop=False)
            # remaining k accumulate; split at the col-512 bank boundary
            for k in (1, 2, 3):
                wa = 512 - k * 128
                nc.tensor.matmul(y0[:, k * 128:512], xtr[:, k * 128:(k + 1) * 128],
                                 hcr[:, 0:wa], start=False, stop=False)
                nc.tensor.matmul(y1[:, 0:k * 128], xtr[:, k * 128:(k + 1) * 128],
                                 hcr[:, wa:512], start=False, stop=(k == 3))
            for k in (5, 6, 7):
                wa = min(512, 1024 - k * 128)
                nc.tensor.matmul(y1[:, (k - 4) * 128:(k - 4) * 128 + wa],
                                 xtr[:, k * 128:(k + 1) * 128],
                                 hcr[:, 0:wa], start=False, stop=(k == 7))
            y = work.tile([P, FB], DT, tag="y")
            nc.vector.tensor_copy(out=y[:, 0:512], in_=y0[:])
            nc.vector.tensor_copy(out=y[:, 512:1024], in_=y1[:])
            nc.sync.dma_start(d_out[:, sl], y[:])

        from collections import deque
        pend = deque()
        for g in range(NGRP):
            pend.append((g, front(g)))
            if len(pend) > 1:
                back(*pend.popleft())
        while pend:
            back(*pend.popleft())
    nc.compile()
    return nc


_CACHE = {}
_TRACE = False
_LAST_RES = None


def kernel(**inputs):
    noise = np.asarray(inputs["noise"], dtype=F32)
    scal = {k: float(np.asarray(v)) for k, v in inputs.items() if k != "noise"}
    in_maps, meta, patches = _host_precompute(scal, noise)

    key = f"nc{meta['trans_chunks']}"
    if key not in _CACHE:
        _CACHE[key] = _build_kernel(meta["gamma"], meta["trans_chunks"])
    nc = _CACHE[key]

    from concourse.bass_utils import run_bass_kernel_spmd
    res = run_bass_kernel_spmd(nc, in_maps, list(range(NCORE)), trace=_TRACE)
    globals()["_LAST_RES"] = res
    out = np.empty((NCORE, 128, NGRP, BLOCK), F32)
    for c in range(NCORE):
        out[c] = res.results[c]["out"].reshape(128, NGRP, BLOCK)
    full = out.transpose(0, 2, 1, 3).reshape(-1)[:N]
    for s0, vals in patches:
        full[s0:s0 + len(vals)] = vals
    return full[None, :]


# revision 19
# speedup vs baseline: 2.7233x; 1.6867x over previous
"""Trainium2 Bass kernel for nn_DifferentiableSynth.

Self-contained: takes FULL inputs (15 scalars + noise[14.4M]), returns [1, 14.4M].
Strategy: shard time axis across 8 cores (1792 blocks of 1024 each, partition=block
layout [128, 14 chunks x 1024]). Host computes O(1) scalars, O(N/16) cumsum row
carries (bit-exact f32 emulation of XLA's blocked cumsum), and exact f32 emulation
of the ~163 transient/release blocks (head ADSR + tail release), which are
overwritten after the gather; device computes all steady-state O(N) work: phase
quantization + range reduction + ACT sine, source mix, then the per-block biquad
as a 512-tap truncated-FIR via PE transposes + fp32r Toeplitz matmuls (amp and
(1-mix) gains folded into the FIR taps).
"""
import numpy as np

SR = 48000
DUR = 300.0
N = 14400000
BLOCK = 1024
NBLK = 14063                 # real blocks (ceil(N/1024))
NCORE = 8
NGRP = 14                    # chunks (groups) per core
GBLK = 128                   # blocks per chunk = partitions
CBLK = NGRP * GBLK           # 1792 blocks per core
CSAMP = CBLK * BLOCK         # 1,835,008 samples per core
TOTBLK = NCORE * CBLK        # 14336 padded blocks
NROW = N // 16               # 900000 rows of 16
ROWS_PC = CSAMP // 16        # 114688 rows per core -> [128, 896]
F32 = np.float32
TAPS = 512                   # FIR truncation (pole radius^512 << tol)
NLAG = TAPS // 128           # 4 lag sub-matrices

MAGIC = np.float32(12582912.0)       # 1.5*2^23
C2PI = np.float32(6.2831855)         # fl32(2*pi)
INV2PI = np.float32(1.0 / (2.0 * np.pi))


def _serial_scan_rows(x2d):
    out = np.empty_like(x2d)
    acc = np.zeros(x2d.shape[0], dtype=F32)
    for j in range(x2d.shape[1]):
        acc = (acc + x2d[:, j]).astype(F32)
        out[:, j] = acc
    return out


def _xla_cumsum_full(x, base=16):
    n = x.shape[0]
    xp = np.pad(x, (0, (-n) % base))
    inner = _serial_scan_rows(xp.reshape(-1, base))
    sums = inner[:, -1].copy()
    if sums.shape[0] <= base:
        outer = _serial_scan_rows(sums[None, :])[0]
    else:
        _, outer = _xla_cumsum_full(sums, base)
    outer_excl = np.concatenate([np.zeros(1, F32), outer[:-1]])
    full = (inner + outer_excl[:, None]).astype(F32).reshape(-1)[:n]
    return inner, full


def _adsr64(a_s, d_s, sus, r_s, idx):
    a = a_s * SR; d = d_s * SR; r = r_s * SR
    attack_end = a; decay_end = a + d; sustain_end = max(decay_end, N - r)
    t = idx.astype(np.float64)
    env = np.full(t.shape, sus)
    env = np.where(t < decay_end, 1.0 - (t - attack_end) / max(d, 1e-5) * (1.0 - sus), env)
    env = np.where(t >= sustain_end, sus * (1.0 - (t - sustain_end) / max(r, 1e-5)), env)
    env = np.where(t < attack_end, t / max(a, 1e-5), env)
    return np.clip(env, 0.0, 1.0)


def _biquad_coeffs(cutoff, q):
    """f64 normalized biquad low-pass coeffs (b0,b1,b2,a1,a2), a0 += 1e-8."""
    w0 = 2.0 * np.pi * cutoff / SR
    alpha_f = np.sin(w0) / (2.0 * q)
    cosw = np.cos(w0)
    b0 = (1.0 - cosw) / 2.0
    a0e = 1.0 + alpha_f + 1e-8
    return (b0 / a0e, (1.0 - cosw) / a0e, b0 / a0e,
            (-2.0 * cosw) / a0e, (1.0 - alpha_f) / a0e)


def _host_precompute(scal, noise):
    """All host-side preparation. Returns per-core input dicts + meta + patches."""
    import jax
    import jax.numpy as jnp
    cpu = jax.devices("cpu")[0]

    with jax.default_device(cpu):
        s = {k: jnp.float32(v) for k, v in scal.items()}
        sg = jax.nn.sigmoid

        def sc(v, dmin, dmax):
            return np.asarray((v - 0.0) / (1.0 - 0.0) * (dmax - dmin) + dmin)

        noise_mix = float(np.asarray(sg(s["noise_mix_raw"])))
        start_freq = np.asarray(sc(sg(s["start_freq_raw"]), 20.0, 8000.0))
        end_freq = np.asarray(sc(sg(s["end_freq_raw"]), 20.0, 8000.0))
        pitch_decay = np.asarray(sc(sg(s["pitch_decay_raw"]), 0.01, 2.0))
        amp_attack = float(np.asarray(sc(sg(s["amp_attack_raw"]), 0.001, 1.0)))
        amp_decay = float(np.asarray(sc(sg(s["amp_decay_raw"]), 0.01, 2.0)))
        amp_sustain = float(np.asarray(sg(s["amp_sustain_raw"])))
        amp_release = float(np.asarray(sc(sg(s["amp_release_raw"]), 0.01, 2.0)))
        cutoff_base = float(np.asarray(sc(sg(s["filter_cutoff_raw"]), 100.0, 12000.0)))
        filter_q = float(np.asarray(sc(sg(s["filter_q_raw"]), 0.707, 10.0)))
        env_amount = float(np.asarray(
            (jnp.tanh(s["filter_env_amount_raw"]) - (-1.0)) / 2.0 * 16000.0 + (-8000.0)))
        fe_attack = float(np.asarray(sc(sg(s["filt_env_attack_raw"]), 0.001, 1.0)))
        fe_decay = float(np.asarray(sc(sg(s["filt_env_decay_raw"]), 0.01, 1.0)))
        fe_sustain = float(np.asarray(sg(s["filt_env_sustain_raw"])))
        fe_release = float(np.asarray(sc(sg(s["filt_env_release_raw"]), 0.01, 1.0)))

        # ---- transient f values (f32 chain, f64 exp) ----
        tau32 = (np.asarray(pitch_decay).astype(F32) + F32(1e-6)).astype(F32)
        tau = float(tau32)
        i_star = int(np.ceil(-np.log(2.0 ** -26) * tau * (N - 1) / DUR)) + 4096
        trans_chunks = min(NGRP, (i_star // (GBLK * BLOCK)) + 1)
        TRANS = trans_chunks * GBLK * BLOCK
        t_f32 = np.asarray(jnp.linspace(0.0, DUR, N)[:TRANS])

    arg = (-t_f32 / tau32).astype(F32)
    pc = np.exp(arg.astype(np.float64)).astype(F32)
    one_m = (F32(1.0) - pc).astype(F32)
    dfreq = (end_freq.astype(F32) - start_freq.astype(F32)).astype(F32)
    freq_tr = (start_freq.astype(F32) + (dfreq * one_m).astype(F32)).astype(F32)
    f_tr = ((C2PI * freq_tr).astype(F32) / F32(SR)).astype(F32)
    cval = ((C2PI * (start_freq.astype(F32) + dfreq).astype(F32)).astype(F32)
            / F32(SR)).astype(F32)

    # ---- cumsum carries: row-of-16 inner prefixes + exclusive row carries ----
    f_full = np.full(N, cval, dtype=F32)
    f_full[:TRANS] = f_tr
    xp = f_full.reshape(-1, 16)
    inner0 = _serial_scan_rows(xp)               # [900000, 16]
    sums0 = inner0[:, -1].copy()
    _, S1 = _xla_cumsum_full(sums0)              # inclusive scan of row sums
    S1x = np.concatenate([np.zeros(1, F32), S1[:-1]])   # exclusive row carries

    # pad rows to 8*114688
    ROWS_TOT = NCORE * ROWS_PC
    S1x_pad = np.concatenate([S1x, np.zeros(ROWS_TOT - NROW, F32)])
    inner0_pad = np.concatenate([inner0, np.zeros((ROWS_TOT - NROW, 16), F32)], 0)

    # r_row (range-reduced carries, f64 precision) and binade magic per (p, g)
    S64 = S1x_pad.astype(np.float64)
    m_int = np.round(S64 * (1.0 / (2 * np.pi)) - 0.05)
    r_row = (S64 - m_int * (2 * np.pi)).astype(F32)

    # per-core row layout [128, 896]: row R(c,p,g,w) = ((c*14+g)*128+p)*64+w
    r4 = r_row.reshape(NCORE, NGRP, GBLK, 64)
    rrow_pc = np.ascontiguousarray(r4.transpose(0, 2, 1, 3)).reshape(NCORE, 128, NGRP * 64)
    S4 = S1x_pad.reshape(NCORE, NGRP, GBLK, 64)
    Sfirst = S4[:, :, :, 0]                       # [core, g, p]
    bits = Sfirst.view(np.uint32) if Sfirst.dtype == F32 else Sfirst.astype(F32).view(np.uint32)
    binade = (bits & np.uint32(0x7F800000)).view(F32)
    Mpg = (F32(1.5) * binade).astype(F32).transpose(0, 2, 1)   # [core, p, g]
    Mpg = np.ascontiguousarray(Mpg)

    # I0 for transient chunks (per-sample inner prefixes)
    tcn = max(2, trans_chunks)
    i4 = inner0_pad.reshape(NCORE, NGRP, GBLK, 64, 16)
    I0 = np.ascontiguousarray(i4[:, 0:tcn].transpose(0, 2, 1, 3, 4)).reshape(NCORE, 128, tcn * 1024)

    # const-row inner prefix pattern (16-periodic)
    P0 = np.zeros(16, F32)
    acc = F32(0.0)
    for j in range(16):
        acc = F32(acc + cval)
        P0[j] = acc

    # Pre-round inner prefixes to the carry binade grid on host (exact f32):
    # t1 = fl(fl(pref + M) - M) with M = Mpg[c, p, g].
    # i0r: per-sample for transient chunks; t1p: 16-periodic pattern per (p, g).
    I0r = np.empty_like(I0)
    T1p = np.empty((NCORE, 128, NGRP * 16), F32)
    for c in range(NCORE):
        for g in range(NGRP):
            M = Mpg[c][:, g:g + 1]
            if g < tcn:
                blk = I0[c][:, g * 1024:(g + 1) * 1024]
                I0r[c][:, g * 1024:(g + 1) * 1024] = \
                    ((blk + M).astype(F32) - M).astype(F32)
            T1p[c][:, g * 16:(g + 1) * 16] = \
                ((P0[None, :] + M).astype(F32) - M).astype(F32)

    # ---- sustain-region FIR taps (amp + (1-mix) folded in) ----
    alpha_mix = 1.0 - noise_mix
    gamma = noise_mix / alpha_mix
    fe_sus_cut = np.clip(cutoff_base + fe_sustain * env_amount, 20.0, SR / 2.1)
    b0n, b1n, b2n, a1n, a2n = _biquad_coeffs(fe_sus_cut, filter_q)
    h = np.zeros(TAPS, np.float64)
    y1 = y2 = 0.0
    for n in range(TAPS):
        fir = (b0n if n == 0 else 0.0) + (b1n if n == 1 else 0.0) + (b2n if n == 2 else 0.0)
        y = fir - a1n * y1 - a2n * y2
        h[n] = y
        y2 = y1; y1 = y
    h *= alpha_mix * amp_sustain
    # Hcat[tau, 128*l + t] = h[128*l + t - tau] (0 for negative lag)
    import ml_dtypes
    BF16 = ml_dtypes.bfloat16
    lag = (np.arange(TAPS)[None, :] - np.arange(128)[:, None])   # [tau, col]
    val = np.where((lag >= 0) & (lag < TAPS), h[np.clip(lag, 0, TAPS - 1)], 0.0)
    Hcat = val.astype(BF16)
    eye = np.eye(128, dtype=BF16)

    # ---- special (non-sustain) blocks: exact f32 host emulation ----
    dec_end_b = int((fe_attack + fe_decay) * SR // BLOCK) + 2
    amp_dec_end_b = int((amp_attack + amp_decay) * SR // BLOCK) + 2
    n_head = min(GBLK, max(dec_end_b, amp_dec_end_b, 2))
    amp_rel_start_b = int((N - amp_release * SR) // BLOCK)
    fe_rel_start_b = int((N - fe_release * SR) // BLOCK)
    tail_start = min(amp_rel_start_b, fe_rel_start_b, NBLK - 1)
    head_blocks = list(range(0, n_head))
    tail_blocks = list(range(tail_start, NBLK))

    def emulate(blist):
        nb = len(blist)
        barr = np.array(blist, np.int64)
        rows = (barr[:, None] * 64 + np.arange(64)[None, :]).reshape(-1)   # rows of 16
        ph32 = (S1x_pad[rows][:, None] + inner0_pad[rows]).astype(F32).reshape(nb, BLOCK)
        sine = np.sin(ph32.astype(np.float64)).astype(F32)
        nzb = np.zeros((nb, BLOCK), F32)
        for i, b in enumerate(blist):
            s0, s1 = b * BLOCK, min((b + 1) * BLOCK, N)
            nzb[i, :s1 - s0] = noise[s0:s1]
        src = ((F32(alpha_mix) * sine).astype(F32) + (F32(noise_mix) * nzb).astype(F32)).astype(F32)
        # per-block cutoff mean (f64 adsr; pad-aware for last block)
        co = np.empty(nb, np.float64)
        for i, b in enumerate(blist):
            idx = np.arange(b * BLOCK, (b + 1) * BLOCK)
            fe = _adsr64(fe_attack, fe_decay, fe_sustain, fe_release, idx)
            cut = np.clip(cutoff_base + fe * env_amount, 20.0, SR / 2.1)
            cut = np.where(idx < N, cut, 0.0)       # reference pads cutoff with 0
            co[i] = cut.mean()
        cb0, cb1, cb2, ca1, ca2 = _biquad_coeffs(co, filter_q)
        cb0 = cb0.astype(F32)[:, None]; cb1 = cb1.astype(F32)[:, None]
        cb2 = cb2.astype(F32)[:, None]
        ca1 = ca1.astype(F32); ca2 = ca2.astype(F32)
        x1 = np.zeros_like(src); x1[:, 1:] = src[:, :-1]
        x2 = np.zeros_like(src); x2[:, 2:] = src[:, :-2]
        fir = ((cb0 * src).astype(F32) + (cb1 * x1).astype(F32)).astype(F32)
        fir = (fir + (cb2 * x2).astype(F32)).astype(F32)
        y = np.zeros((nb, BLOCK), F32)
        yy1 = np.zeros(nb, F32); yy2 = np.zeros(nb, F32)
        for t in range(BLOCK):
            v = ((fir[:, t] - (ca1 * yy1).astype(F32)).astype(F32)
                 - (ca2 * yy2).astype(F32)).astype(F32)
            y[:, t] = v
            yy2 = yy1; yy1 = v
        for i, b in enumerate(blist):
            idx = np.arange(b * BLOCK, (b + 1) * BLOCK)
            amp = _adsr64(amp_attack, amp_decay, amp_sustain, amp_release, idx).astype(F32)
            y[i] = (y[i] * amp).astype(F32)
        return y

    patches = []
    for blist in (head_blocks, tail_blocks):
        if not blist:
            continue
        yv = emulate(blist)
        for i, b in enumerate(blist):
            s0, s1 = b * BLOCK, min((b + 1) * BLOCK, N)
            patches.append((s0, yv[i, :s1 - s0]))

    # noise shards [core, 128, 14336], pre-scaled by gamma, bf16
    noise_pad = np.concatenate([noise.astype(F32), np.zeros(TOTBLK * BLOCK - N, F32)])
    gnz = (noise_pad * F32(gamma)).astype(F32).astype(BF16)
    nz = np.ascontiguousarray(
        gnz.reshape(NCORE, NGRP, GBLK, BLOCK).transpose(0, 2, 1, 3)
    ).reshape(NCORE, 128, NGRP * BLOCK)

    in_maps = []
    for c in range(NCORE):
        in_maps.append({
            "nz": nz[c],
            "rrow": np.ascontiguousarray(rrow_pc[c]),
            "t1p": np.ascontiguousarray(T1p[c]),
            "i0": np.ascontiguousarray(I0r[c]),
            "hcat": Hcat,
            "eye": eye,
        })
    meta = {"gamma": gamma, "trans_chunks": tcn}
    return in_maps, meta, patches


def _build_kernel(gamma, trans_chunks):
    from contextlib import ExitStack
    import concourse.bass as bass
    import concourse.tile as tile
    from concourse import bacc, mybir

    A = mybir.AluOpType
    DT = mybir.dt.float32
    BF = mybir.dt.bfloat16
    P = 128
    FB = BLOCK

    nc = bacc.Bacc("TRN2", target_bir_lowering=False, debug=False, num_devices=NCORE)
    d_nz = nc.dram_tensor("nz", [P, NGRP * FB], BF, kind="ExternalInput").ap()
    d_rrow = nc.dram_tensor("rrow", [P, NGRP * 64], DT, kind="ExternalInput").ap()
    d_t1p = nc.dram_tensor("t1p", [P, NGRP * 16], DT, kind="ExternalInput").ap()
    d_i0 = nc.dram_tensor("i0", [P, trans_chunks * FB], DT, kind="ExternalInput").ap()
    d_hcat = nc.dram_tensor("hcat", [P, TAPS], BF, kind="ExternalInput").ap()
    d_eye = nc.dram_tensor("eye", [P, P], BF, kind="ExternalInput").ap()
    d_out = nc.dram_tensor("out", [P, NGRP * FB], DT, kind="ExternalOutput").ap()

    with tile.TileContext(nc) as tc, ExitStack() as ctx:
        statics = ctx.enter_context(tc.tile_pool(name="static", bufs=1))
        work = ctx.enter_context(tc.tile_pool(name="work", bufs=3))
        psum = ctx.enter_context(tc.tile_pool(name="ps", bufs=2, space="PSUM"))

        rrow = statics.tile([P, NGRP * 64], DT)
        t1p = statics.tile([P, NGRP * 16], DT)
        i0t = statics.tile([P, trans_chunks * FB], DT)
        hcat = statics.tile([P, TAPS], BF)
        eye = statics.tile([P, P], BF)
        nc.sync.dma_start(rrow[:], d_rrow[:])
        nc.sync.dma_start(t1p[:], d_t1p[:])
        nc.sync.dma_start(i0t[:], d_i0[:])
        nc.sync.dma_start(hcat[:], d_hcat[:])
        nc.sync.dma_start(eye[:], d_eye[:])
        negmagict = statics.tile([P, 1], DT)
        nc.vector.memset(negmagict[:], -float(MAGIC))

        def front(g):
            sl = slice(g * FB, (g + 1) * FB)
            nz = work.tile([P, FB], BF, tag="nz")
            nc.sync.dma_start(nz[:], d_nz[:, sl])
            # ph = pre-rounded inner prefix + range-reduced row carry
            ph = work.tile([P, FB], DT, tag="ph")
            rb_ap = rrow[:, g * 64:(g + 1) * 64].rearrange(
                "p (w j) -> p w j", j=1).broadcast_to([P, 64, 16])
            if g < trans_chunks:
                in0 = i0t[:, sl].rearrange("p (w j) -> p w j", w=64)
            else:
                in0 = t1p[:, g * 16:(g + 1) * 16].rearrange(
                    "p (w j) -> p w j", w=1).broadcast_to([P, 64, 16])
            nc.gpsimd.tensor_tensor(
                ph[:].rearrange("p (w j) -> p w j", w=64), in0, rb_ap, A.add)
            qp = work.tile([P, FB], DT, tag="qp")
            nc.scalar.activation(qp[:], ph[:], mybir.ActivationFunctionType.Copy,
                                 bias=float(MAGIC), scale=float(INV2PI))
            nc.scalar.activation(qp[:], qp[:], mybir.ActivationFunctionType.Identity,
                                 bias=negmagict[:])
            # nsin = (m * 2pi) - ph = -p1 ; Sin with scale=-1 restores sign
            nsin = work.tile([P, FB], DT, tag="nsin")
            nc.vector.scalar_tensor_tensor(nsin[:], qp[:], float(C2PI),
                                           ph[:], A.mult, A.subtract)
            sine = work.tile([P, FB], DT, tag="sine")
            nc.scalar.activation(sine[:], nsin[:], mybir.ActivationFunctionType.Sin,
                                 scale=-1.0)
            src = work.tile([P, FB], BF, tag="src")
            nc.vector.tensor_tensor(src[:], sine[:], nz[:], A.add)
            return src

        def back(g, src):
            sl = slice(g * FB, (g + 1) * FB)
            xt_ps = psum.tile([P, FB], BF, tag="xt")
            for k in range(8):
                nc.tensor.transpose(xt_ps[:, k * 128:(k + 1) * 128],
                                    src[:, k * 128:(k + 1) * 128], eye[:])
            xt = work.tile([P, FB], BF, tag="xtsb")
            nc.vector.tensor_copy(out=xt[:], in_=xt_ps[:])
            xtr = xt[:]
            hcr = hcat[:]
            y_ps = psum.tile([P, FB], DT, tag="y")
            # k=0 and k=4 first with start=True (each resets one full bank);
            # remaining k accumulate, APs may span the col-512 bank boundary
            nc.tensor.matmul(y_ps[:, 0:512], xtr[:, 0:128], hcr[:, 0:512],
                             start=True, stop=False)
            nc.tensor.matmul(y_ps[:, 512:1024], xtr[:, 512:640], hcr[:, 0:512],
                             start=True, stop=False)
            for k in (1, 2, 3, 5, 6, 7):
                wa = min(512, 1024 - k * 128)
                nc.tensor.matmul(y_ps[:, k * 128:k * 128 + wa],
                                 xtr[:, k * 128:(k + 1) * 128],
                                 hcr[:, 0:wa], start=False, stop=(k == 7))
            y = work.tile([P, FB], DT, tag="ysb")
            nc.vector.tensor_copy(out=y[:, 0:512], in_=y_ps[:, 0:512])
            nc.scalar.copy(y[:, 512:1024], y_ps[:, 512:1024])
            nc.sync.dma_start(d_out[:, sl], y[:])

        from collections import deque
        pend = deque()
        for g in range(NGRP):
            pend.append((g, front(g)))
            if len(pend) > 1:
                back(*pend.popleft())
        while pend:
            back(*pend.popleft())
    nc.compile()
    return nc


_CACHE = {}
_TRACE = False
_LAST_RES = None


def kernel(**inputs):
    noise = np.asarray(inputs["noise"], dtype=F32)
    scal = {k: float(np.asarray(v)) for k, v in inputs.items() if k != "noise"}
    in_maps, meta, patches = _host_precompute(scal, noise)

    key = f"nc{meta['trans_chunks']}"
    if key not in _CACHE:
        _CACHE[key] = _build_kernel(meta["gamma"], meta["trans_chunks"])
    nc = _CACHE[key]

    from concourse.bass_utils import run_bass_kernel_spmd
    res = run_bass_kernel_spmd(nc, in_maps, list(range(NCORE)), trace=_TRACE)
    globals()["_LAST_RES"] = res
    out = np.empty((NCORE, 128, NGRP, BLOCK), F32)
    for c in range(NCORE):
        out[c] = res.results[c]["out"].reshape(128, NGRP, BLOCK)
    full = out.transpose(0, 2, 1, 3).reshape(-1)[:N]
    for s0, vals in patches:
        full[s0:s0 + len(vals)] = vals
    return full[None, :]
